# revision 8
# baseline (speedup 1.0000x reference)
"""Expert-parallel MoE (top-2 of 8 experts, SwiGLU FFN) for 8 Trainium2 cores.

Strategy (balanced expert-parallel, all-to-all on host):
  - Host computes the small gate (logits -> top-2 -> softmax) in float64
    and dispatches token-expert pairs to cores.
  - Load balancing: instead of padding every core to the hottest expert's
    count (2152 for the graded input), each core runs TWO expert segments
    of fixed sizes (a, b).  The six middle experts each occupy one core's
    both segments; the hottest expert is split across two cores' b-slots
    and the coldest across the same two cores' a-slots.  Max-core load
    drops to max(c2, ceil(c1/2)+ceil(c8/2)) ~= 2068 tokens (vs 2152).
  - All matmuls in bfloat16: 1 cycle/row on the PE with fast-weight-load
    (LDWEIGHTS hidden), vs f32r's ~40cy/matmul weight bubble.  rel err
    ~4e-3, well inside the 2e-2 gate.
  - Single weight pass: h = silu(x@Wg)*(x@Wu) for ALL tokens is kept in
    SBUF in bf16 (22 h-tiles x 2068 cols x 2B = 91KB/partition), so
    Wg/Wu/Wd stream from HBM exactly once per segment-slot.
  - Feature-major layout throughout (features on partitions, tokens on
    the free axis) so h feeds the down-projection without transposes.
  - DMA descriptors are enqueued in critical-path order (first chunk's
    x k=0 slice, then ht=0 weights, then the rest) since the Sync engine
    issues them strictly in program order at ~0.6us apiece.
"""

import numpy as np

DIM = 1024
HID = 2816
E = 8
TOPK = 2
P = 128
KD = DIM // P   # 8 k-subtiles (contraction of x@W)
HT = HID // P   # 22 h-subtiles
DT = DIM // P   # 8 d-subtiles (output features)
CHUNK_MAX = 512  # PSUM bank limit (512 fp32 accumulators)

_KERNEL_CACHE = {}
LAST_RESULTS = None  # BassKernelResults of the most recent run (for profiling)


def _split_chunks(size, off0):
    """Split `size` into ceil(size/CHUNK_MAX) near-equal even chunks."""
    if size <= 0:
        return []
    n = -(-size // CHUNK_MAX)
    base = size // n
    sizes = [base] * n
    for i in range(size - base * n):
        sizes[i] += 1
    # make every chunk even (pairwise shuffle of the odd ones)
    for i in range(n - 1):
        if sizes[i] % 2:
            sizes[i] += 1
            sizes[i + 1] -= 1
    assert sum(sizes) == size and all(0 < s <= CHUNK_MAX for s in sizes)
    chunks = []
    off = off0
    for s in sizes:
        chunks.append((off, s))
        off += s
    return chunks


def _build_moe_ffn(a, b):
    """Per-core Bass program: two expert segments (sizes a and b) of a
    SwiGLU FFN, feature-major, single weight pass per segment."""
    import concourse.bass as bass  # noqa: F401
    import concourse.mybir as mybir
    from concourse import bacc, tile

    f32 = mybir.dt.float32
    bf16 = mybir.dt.bfloat16
    SiLU = mybir.ActivationFunctionType.Silu

    M = a + b
    chunks = [(off, sz, "a") for off, sz in _split_chunks(a, 0)] + [
        (off, sz, "b") for off, sz in _split_chunks(b, a)
    ]

    nc = bacc.Bacc("TRN2", target_bir_lowering=False, debug=False)

    xt = nc.dram_tensor("xt", [P, KD, M], bf16, kind="ExternalInput")
    wgt = {}
    wut = {}
    wdt = {}
    for s in ("a", "b"):
        wgt[s] = nc.dram_tensor(f"wgt_{s}", [HT, P, KD, P], bf16, kind="ExternalInput")
        wut[s] = nc.dram_tensor(f"wut_{s}", [HT, P, KD, P], bf16, kind="ExternalInput")
        wdt[s] = nc.dram_tensor(f"wdt_{s}", [DT, P, HT, P], bf16, kind="ExternalInput")
    yt = nc.dram_tensor("yt", [DT, P, M], f32, kind="ExternalOutput")

    with tile.TileContext(nc) as tc:
        with (
            tc.tile_pool(name="xp", bufs=1) as xp,
            tc.tile_pool(name="wp", bufs=3) as wp,
            tc.tile_pool(name="hp", bufs=1) as hp,
            tc.tile_pool(name="op", bufs=3) as op,
            tc.tile_pool(name="ps", bufs=4, space="PSUM") as ps,
        ):
            # DMA issue order is program order on the Sync engine (~0.6us
            # per descriptor), so the critical path of the very first
            # matmuls is enqueued first: x(k=0) of chunk 0, then the ht=0
            # weights, then everything else.
            x_sb = [None] * len(chunks)
            w_cache = {}

            def load_w(ht, slots="ab"):
                ws = w_cache.setdefault(ht, {})
                for s in slots:
                    wg_sb = wp.tile([P, KD, P], bf16, tag=f"wg{s}",
                                    name=f"wg{s}")
                    nc.sync.dma_start(wg_sb[:], wgt[s][ht])
                    wu_sb = wp.tile([P, KD, P], bf16, tag=f"wu{s}",
                                    name=f"wu{s}")
                    nc.sync.dma_start(wu_sb[:], wut[s][ht])
                    ws[s] = (wg_sb, wu_sb)

            # chunk 0's x arrives as 8 per-k slices interleaved with the
            # ht=0 weights: the opening matmuls consume k-slices slower
            # (~0.4us each, cold) than the DMA delivers them, so the PE
            # starts at the ~4.3us launch floor instead of waiting for a
            # monolithic 1MB x transfer.
            off0, csize0, slot0 = chunks[0]
            x0k = []
            for kt in range(KD):
                xk = xp.tile([P, csize0], bf16, tag=f"x0k{kt}", name=f"x0k{kt}")
                x0k.append(xk)
            nc.sync.dma_start(x0k[0][:], xt[:, 0, off0 : off0 + csize0])
            nc.sync.dma_start(x0k[1][:], xt[:, 1, off0 : off0 + csize0])
            load_w(0, slot0)
            nc.sync.dma_start(x0k[2][:], xt[:, 2, off0 : off0 + csize0])
            nc.sync.dma_start(x0k[3][:], xt[:, 3, off0 : off0 + csize0])
            for kt in range(4, KD):
                nc.sync.dma_start(x0k[kt][:], xt[:, kt, off0 : off0 + csize0])
            load_w(0, "b" if slot0 == "a" else "a")
            x_sb[0] = x0k

            # remaining chunks' x and the ht=1 weights, interleaved in
            # consumption order
            for ci, (off, csize, _) in enumerate(chunks):
                if ci == 0:
                    continue
                xc = xp.tile([P, KD, csize], bf16, tag=f"x{ci}", name=f"x{ci}")
                nc.sync.dma_start(xc[:], xt[:, :, off : off + csize])
                x_sb[ci] = [xc[:, kt] for kt in range(KD)]
                if ci == 1 and HT > 1:
                    load_w(1)

            # --- h = silu(x @ Wg) * (x @ Wu), kept in SBUF (bf16)
            h_sb = hp.tile([P, HT, M], bf16, tag="h")

            for ht in range(HT):
                if ht not in w_cache:
                    load_w(ht)
                ws = w_cache.pop(ht)
                for ci, (off, csize, slot) in enumerate(chunks):
                    wg_sb, wu_sb = ws[slot]
                    pg = ps.tile([P, csize], f32, tag="g", bufs=4)
                    pu = ps.tile([P, csize], f32, tag="u", bufs=4)
                    for kt in range(KD):
                        nc.tensor.matmul(
                            pg,
                            wg_sb[:, kt],
                            x_sb[ci][kt],
                            start=(kt == 0),
                            stop=(kt == KD - 1),
                        )
                    for kt in range(KD):
                        nc.tensor.matmul(
                            pu,
                            wu_sb[:, kt],
                            x_sb[ci][kt],
                            start=(kt == 0),
                            stop=(kt == KD - 1),
                        )
                    sl = op.tile([P, csize], f32, tag="silu")
                    nc.scalar.activation(sl[:], pg, SiLU)
                    nc.vector.tensor_mul(
                        h_sb[:, ht, off : off + csize], sl[:], pu
                    )

            # --- y = h @ Wd, feature-major; chunks largest-first so the
            # kernel-tail copy+DMA drains the smallest chunk
            down_chunks = sorted(chunks, key=lambda c: -c[1])
            for dt in range(DT):
                wd = {}
                for s in ("a", "b"):
                    wd_sb = wp.tile([P, HT, P], bf16, tag=f"wd{s}", bufs=2,
                                    name=f"wd{s}")
                    nc.sync.dma_start(wd_sb[:], wdt[s][dt])
                    wd[s] = wd_sb
                for ci, (off, csize, slot) in enumerate(down_chunks):
                    py = ps.tile([P, csize], f32, tag="g")
                    for ht in range(HT):
                        nc.tensor.matmul(
                            py,
                            wd[slot][:, ht],
                            h_sb[:, ht, off : off + csize],
                            start=(ht == 0),
                            stop=(ht == HT - 1),
                        )
                    o_sb = op.tile([P, csize], f32, tag="o")
                    nc.vector.tensor_copy(o_sb[:], py)
                    nc.sync.dma_start(yt[dt, :, off : off + csize], o_sb[:])

    nc.finalize()
    return nc


def _get_kernel(a, b):
    if (a, b) not in _KERNEL_CACHE:
        _KERNEL_CACHE[(a, b)] = _build_moe_ffn(a, b)
    return _KERNEL_CACHE[(a, b)]


def _route(xf, W_gate):
    """Replicate reference routing: top-2 by logit, softmax weights.

    float64 logits: the top-k decision boundary gap is >> f32 rounding
    noise, so this matches the f32 jax reference's selection."""
    logits = xf.astype(np.float64) @ W_gate.astype(np.float64)  # [N, E]
    order = np.argsort(-logits, axis=1, kind="stable")[:, :TOPK]  # [N, 2]
    top = np.take_along_axis(logits, order, axis=1)
    top = top - top.max(axis=1, keepdims=True)
    ew = np.exp(top)
    w = (ew / ew.sum(axis=1, keepdims=True)).astype(np.float32)  # [N, 2]
    return order, w


def _plan(ids):
    """Balanced assignment of per-expert token lists to 8 cores x 2 slots.

    Returns (a, b, jobs): jobs[core][slot] = (expert, lo, hi) — the
    half-open range of that expert's token list handled by the slot."""
    counts = np.array([len(i) for i in ids])
    desc = np.argsort(-counts, kind="stable")
    c = counts[desc]
    c0h = (int(c[0]) + 1) // 2   # hottest expert, split over two b-slots
    c7h = (int(c[7]) + 1) // 2   # coldest expert, split over two a-slots
    n8 = -(-int(counts.sum()) // 8)
    bb = (c0h + 1) // 2 * 2
    M = max(int(c[1]), bb + c7h, n8)
    M = (M + 1) // 2 * 2
    a = M - bb
    jobs = []
    for i in range(6):
        e = int(desc[i + 1])
        n = int(counts[e])
        na = min(n, a)
        jobs.append({"a": (e, 0, na), "b": (e, na, n)})
    e_hot, e_cold = int(desc[0]), int(desc[7])
    jobs.append({"a": (e_cold, 0, c7h), "b": (e_hot, 0, c0h)})
    jobs.append({"a": (e_cold, c7h, int(c[7])), "b": (e_hot, c0h, int(c[0]))})
    for j in jobs:  # validate capacity
        ea, lo_a, hi_a = j["a"]
        eb, lo_b, hi_b = j["b"]
        assert 0 <= hi_a - lo_a <= a and 0 <= hi_b - lo_b <= bb
    return a, bb, jobs


def _bf16(x):
    import ml_dtypes

    return np.ascontiguousarray(x).astype(ml_dtypes.bfloat16)


def kernel(x, W_gate, Wg, Wu, Wd):
    from concourse.bass_utils import run_bass_kernel_spmd

    x = np.ascontiguousarray(np.asarray(x, dtype=np.float32))
    W_gate = np.asarray(W_gate, dtype=np.float32)
    Wg = np.asarray(Wg, dtype=np.float32)
    Wu = np.asarray(Wu, dtype=np.float32)
    Wd = np.asarray(Wd, dtype=np.float32)

    B, T, D = x.shape
    xf = x.reshape(-1, D)
    N = xf.shape[0]

    order, w = _route(xf, W_gate)

    ids = []  # per-expert token indices
    wts = []  # per-expert combine weights
    for e in range(E):
        sel = np.nonzero(order == e)
        ids.append(sel[0])
        wts.append(w[sel[0], sel[1]])

    a, b, jobs = _plan(ids)
    M = a + b

    nc = _get_kernel(a, b)

    # expert weights in device layout (bf16, feature-major), cached per expert
    w_dev = {}

    def expert_w(e):
        if e not in w_dev:
            wg_t = _bf16(Wg[e].reshape(KD, P, HT, P).transpose(2, 1, 0, 3))
            wu_t = _bf16(Wu[e].reshape(KD, P, HT, P).transpose(2, 1, 0, 3))
            wd_t = _bf16(Wd[e].reshape(HT, P, DT, P).transpose(2, 1, 0, 3))
            w_dev[e] = (wg_t, wu_t, wd_t)
        return w_dev[e]

    in_maps = []
    scat = []  # per core: list of (col_off, global_ids, combine_wts)
    for core in range(E):
        xe = np.zeros((M, DIM), dtype=np.float32)
        sc = []
        for slot, col in (("a", 0), ("b", a)):
            e, lo, hi = jobs[core][slot]
            gids = ids[e][lo:hi]
            cnt = hi - lo
            xe[col : col + cnt] = xf[gids]
            sc.append((col, gids, wts[e][lo:hi]))
        x_t = np.ascontiguousarray(_bf16(xe.T).reshape(KD, P, M).transpose(1, 0, 2))
        ea = jobs[core]["a"][0]
        eb = jobs[core]["b"][0]
        wga, wua, wda = expert_w(ea)
        wgb, wub, wdb = expert_w(eb)
        in_maps.append(
            {
                "xt": x_t,
                "wgt_a": wga,
                "wut_a": wua,
                "wdt_a": wda,
                "wgt_b": wgb,
                "wut_b": wub,
                "wdt_b": wdb,
            }
        )
        scat.append(sc)

    import os

    if os.environ.get("KERNEL_EMULATE"):
        results = _emulate(in_maps, a, b)
        res = None
    else:
        res = run_bass_kernel_spmd(nc, in_maps, core_ids=list(range(E)))
        global LAST_RESULTS
        LAST_RESULTS = res
        results = [r["yt"] for r in res.results]

    out = np.zeros((N, D), dtype=np.float32)
    for core in range(E):
        y_c = results[core].reshape(DIM, M)
        for col, gids, cw in scat[core]:
            cnt = len(gids)
            if cnt:
                out[gids] += cw[:, None] * y_c[:, col : col + cnt].T
    return out.reshape(B, T, D)


def _emulate(in_maps, a, b):
    """Numpy emulation of the device program (for host-logic testing)."""
    M = a + b
    outs = []
    for im in in_maps:
        x = (
            np.asarray(im["xt"], dtype=np.float32)
            .transpose(1, 0, 2)
            .reshape(DIM, M)
        )  # feature-major [D, M]
        y = np.zeros((DIM, M), dtype=np.float32)
        for slot, col, size in (("a", 0, a), ("b", a, b)):
            wg = (
                np.asarray(im[f"wgt_{slot}"], dtype=np.float32)
                .transpose(2, 1, 0, 3)
                .reshape(DIM, HID)
            )
            wu = (
                np.asarray(im[f"wut_{slot}"], dtype=np.float32)
                .transpose(2, 1, 0, 3)
                .reshape(DIM, HID)
            )
            wd = (
                np.asarray(im[f"wdt_{slot}"], dtype=np.float32)
                .transpose(2, 1, 0, 3)
                .reshape(HID, DIM)
            )
            xs = x[:, col : col + size]
            g = wg.T @ xs
            u = wu.T @ xs
            import ml_dtypes

            h = ((g / (1 + np.exp(-g))) * u).astype(ml_dtypes.bfloat16)
            y[:, col : col + size] = wd.T @ h.astype(np.float32)
        outs.append(y.reshape(DT, P, M))
    return outs
